# revision 12
# baseline (speedup 1.0000x reference)
# Trainium2 Bass kernel for nn_Ml4fTransformer_48421461295652.
#
# Mathematical note (exact, architecture-level dead-code elimination):
# The decoder feature dim DD == 1, so every decoder LayerNorm normalizes over a
# single element: mean(x) == x exactly, so (x - mu) == 0 exactly, var == 0, and
# LN(x, g, b) == 0 * rsqrt(eps) * g + b == b, *exactly*, in any float precision
# and for ANY input values. In particular the final decoder LayerNorm output
# dec_out is dec_norm_b broadcast to (B, PRED) = (16, 64). Hence the reference
# output is
#     out[b, j] = relu(sum_k dec_norm_b[0] * map_w[k, j] + map_b[j])
# for all b — independent of x, y, the whole encoder stack, the learn layer and
# every other weight. This identity holds for any inputs of these shapes, so
# computing it directly is an exact program transformation (verified against
# the full fp32 reference on the spec inputs and on fully randomized inputs:
# rel err ~1e-7, fp32 summation-order noise only).
#
# Sharding strategy: the live computation is a 64x64 reduction + pointwise —
# microseconds of work, entirely fixed-overhead-bound. The live operands
# (map_w, map_b, dec_norm_b) are marshalled into one (65, 65) array, replicated
# to all 8 NeuronCores, and the identical tiny kernel runs SPMD on cores 0-7
# (per-core compute, no collectives). Each core emits the unique [1, 64] row;
# the unshard step broadcasts it to the (16, 64) full output (all 16 batch
# rows are mathematically identical).
#
# Host-side packing (layout only, no arithmetic):
#   packed[0:64, 0:64] = map_w                (partition k, free j)
#   packed[64, 0:64]   = map_b
#   packed[0:64, 64]   = dec_norm_b[0]        (c replicated down a column)
#   packed[64, 64]     = 1.0                  (constant lhsT entry for the b-add)
#
# On-device computation (per core), all fp32 — 4 instructions:
#   T[65,65] <- one DMA of packed
#   S[1,64]   = matmul(lhsT=T[:,64:65], rhs=T[:,0:64])  # K=65 contraction:
#               = sum_k c*map_w[k,j] + 1.0*map_b[j]     #   scale, sum AND bias
#   row[1,64] = max(S, 0)                               # ReLU (DVE)
#   DMA row -> DRAM
# (The matmul reproduces the reference's own contraction order
#  sum_k dec_out[b,k]*map_w[k,j] with dec_out[b,k] == c, plus the bias row.)

import numpy as np

_B, _PRED = 16, 64
_N_CORES = 8

_cached = None  # compiled Bass module — compile once per process


def _build_nc():
    import concourse.mybir as mybir
    import concourse.tile as tile
    from concourse import bacc

    class _LeanBacc(bacc.Bacc):
        # Bass.__init__ unconditionally emits four const-AP memsets plus an
        # all-engine barrier before user code. This kernel never reads the
        # const APs, and on a ~14us kernel that barrier measurably delays the
        # input DMA. Skip only the barrier emitted during construction; every
        # later call (Tile's exit drain/sem-reset barriers, which hardware
        # requires for clean NEFF completion — skipping the final one crashes
        # the exec unit, measured) goes through unchanged.
        _in_ctor = True

        def all_engine_barrier(self, *a, **k):
            if self._in_ctor:
                return None
            return super().all_engine_barrier(*a, **k)

    fp32 = mybir.dt.float32
    nc = _LeanBacc("TRN2", target_bir_lowering=False, debug=False)
    nc._in_ctor = False  # instance attr shadows the class flag from here on

    p_d = nc.dram_tensor("packed", [65, 65], fp32, kind="ExternalInput")
    o_d = nc.dram_tensor("out", [1, _PRED], fp32, kind="ExternalOutput")

    with tile.TileContext(nc) as tc:
        with (
            tc.tile_pool(name="sbuf", bufs=1) as pool,
            tc.tile_pool(name="psum", bufs=1, space="PSUM") as psum,
        ):
            T = pool.tile([65, 65], fp32)
            nc.sync.dma_start(T[:], p_d[:])

            S = psum.tile([1, _PRED], fp32)
            # single K=65 contraction: S = sum_k c*W[k,j] + 1.0*map_b[j]
            nc.tensor.matmul(S[:], T[:, 64:65], T[:, :64],
                             start=True, stop=True)

            row = pool.tile([1, _PRED], fp32)
            nc.vector.tensor_scalar_max(row[:], S[:], 0.0)

            nc.sync.dma_start(o_d[:], row[:])

    nc.compile()
    return nc


def _get_nc():
    global _cached
    if _cached is None:
        _cached = _build_nc()
    return _cached


def _pack(inputs):
    packed = np.empty((65, 65), dtype=np.float32)
    packed[:64, :64] = np.asarray(inputs["map_w"], dtype=np.float32)
    packed[64, :64] = np.asarray(inputs["map_b"], dtype=np.float32).reshape(64)
    packed[:64, 64] = np.asarray(inputs["dec_norm_b"], dtype=np.float32).reshape(())
    packed[64, 64] = 1.0
    return packed


def _run(inputs, trace=False, **kw):
    from concourse.bass_utils import run_bass_kernel_spmd

    nc = _get_nc()
    in_map = {"packed": _pack(inputs)}
    in_maps = [in_map for _ in range(_N_CORES)]
    try:
        return run_bass_kernel_spmd(nc, in_maps, core_ids=list(range(_N_CORES)),
                                    trace=trace, **kw)
    except Exception:
        # one retry — transient device-state failures (e.g. a previous process
        # crashed mid-execution and left a core wedged) clear on re-run
        return run_bass_kernel_spmd(nc, in_maps, core_ids=list(range(_N_CORES)),
                                    trace=trace, **kw)


def _unshard(res):
    row = np.asarray(res.results[0]["out"], dtype=np.float32).reshape(1, _PRED)
    return np.ascontiguousarray(np.broadcast_to(row, (_B, _PRED)))


def kernel(**inputs) -> np.ndarray:
    return _unshard(_run(inputs, trace=False))


# revision 13
# speedup vs baseline: 1.0455x; 1.0455x over previous
# Trainium2 Bass kernel for nn_Ml4fTransformer_48421461295652.
#
# Mathematical note (exact, architecture-level dead-code elimination):
# The decoder feature dim DD == 1, so every decoder LayerNorm normalizes over a
# single element: mean(x) == x exactly, so (x - mu) == 0 exactly, var == 0, and
# LN(x, g, b) == 0 * rsqrt(eps) * g + b == b, *exactly*, in any float precision
# and for ANY input values. In particular the final decoder LayerNorm output
# dec_out is dec_norm_b broadcast to (B, PRED) = (16, 64). Hence the reference
# output is
#     out[b, j] = relu(sum_k dec_norm_b[0] * map_w[k, j] + map_b[j])
# for all b — independent of x, y, the whole encoder stack, the learn layer and
# every other weight. This identity holds for any inputs of these shapes, so
# computing it directly is an exact program transformation (verified against
# the full fp32 reference on the spec inputs and on fully randomized inputs:
# rel err ~1e-7, fp32 summation-order noise only).
#
# Sharding strategy: the live computation is a 64x64 reduction + pointwise —
# microseconds of work, entirely fixed-overhead-bound. The live operands
# (map_w, map_b, dec_norm_b) are marshalled into one (65, 65) array, replicated
# to all 8 NeuronCores, and the identical tiny kernel runs SPMD on cores 0-7
# (per-core compute, no collectives). Each core emits the unique [1, 64] row;
# the unshard step broadcasts it to the (16, 64) full output (all 16 batch
# rows are mathematically identical).
#
# Host-side packing (layout only, no arithmetic):
#   packed[0:64, 0:64] = map_w                (partition k, free j)
#   packed[64, 0:64]   = map_b
#   packed[0:64, 64]   = dec_norm_b[0]        (c replicated down a column)
#   packed[64, 64]     = 1.0                  (constant lhsT entry for the b-add)
#
# On-device computation (per core), all fp32 — 4 instructions:
#   T[65,65] <- one DMA of packed
#   S[1,64]   = matmul(lhsT=T[:,64:65], rhs=T[:,0:64])  # K=65 contraction:
#               = sum_k c*map_w[k,j] + 1.0*map_b[j]     #   scale, sum AND bias
#   row[1,64] = max(S, 0)                               # ReLU (DVE)
#   DMA row -> DRAM
# (The matmul reproduces the reference's own contraction order
#  sum_k dec_out[b,k]*map_w[k,j] with dec_out[b,k] == c, plus the bias row.)

import numpy as np

_B, _PRED = 16, 64
_N_CORES = 8

_cached = None  # compiled Bass module — compile once per process


def _build_nc():
    import concourse.mybir as mybir
    import concourse.tile as tile
    from concourse import bacc

    class _LeanBacc(bacc.Bacc):
        # Bass.__init__ unconditionally emits four const-AP memsets plus an
        # all-engine barrier before user code. This kernel never reads the
        # const APs, and on a ~14us kernel that barrier measurably delays the
        # input DMA. Skip only the barrier emitted during construction; every
        # later call (Tile's exit drain/sem-reset barriers, which hardware
        # requires for clean NEFF completion — skipping the final one crashes
        # the exec unit, measured) goes through unchanged.
        _in_ctor = True

        def all_engine_barrier(self, *a, **k):
            if self._in_ctor:
                return None
            return super().all_engine_barrier(*a, **k)

    fp32 = mybir.dt.float32
    nc = _LeanBacc("TRN2", target_bir_lowering=False, debug=False)
    nc._in_ctor = False  # instance attr shadows the class flag from here on

    p_d = nc.dram_tensor("packed", [65, 65], fp32, kind="ExternalInput")
    o_d = nc.dram_tensor("out", [1, _PRED], fp32, kind="ExternalOutput")

    with tile.TileContext(nc) as tc:
        with (
            tc.tile_pool(name="sbuf", bufs=1) as pool,
            tc.tile_pool(name="psum", bufs=1, space="PSUM") as psum,
        ):
            T = pool.tile([65, 65], fp32)
            nc.scalar.dma_start(T[:], p_d[:])

            S = psum.tile([1, _PRED], fp32)
            # single K=65 contraction: S = sum_k c*W[k,j] + 1.0*map_b[j]
            nc.tensor.matmul(S[:], T[:, 64:65], T[:, :64],
                             start=True, stop=True)

            row = pool.tile([1, _PRED], fp32)
            nc.vector.tensor_scalar_max(row[:], S[:], 0.0)

            nc.sync.dma_start(o_d[:], row[:])

    nc.compile()
    return nc


def _get_nc():
    global _cached
    if _cached is None:
        _cached = _build_nc()
    return _cached


def _pack(inputs):
    packed = np.empty((65, 65), dtype=np.float32)
    packed[:64, :64] = np.asarray(inputs["map_w"], dtype=np.float32)
    packed[64, :64] = np.asarray(inputs["map_b"], dtype=np.float32).reshape(64)
    packed[:64, 64] = np.asarray(inputs["dec_norm_b"], dtype=np.float32).reshape(())
    packed[64, 64] = 1.0
    return packed


def _run(inputs, trace=False, **kw):
    from concourse.bass_utils import run_bass_kernel_spmd

    nc = _get_nc()
    in_map = {"packed": _pack(inputs)}
    in_maps = [in_map for _ in range(_N_CORES)]
    try:
        return run_bass_kernel_spmd(nc, in_maps, core_ids=list(range(_N_CORES)),
                                    trace=trace, **kw)
    except Exception:
        # one retry — transient device-state failures (e.g. a previous process
        # crashed mid-execution and left a core wedged) clear on re-run
        return run_bass_kernel_spmd(nc, in_maps, core_ids=list(range(_N_CORES)),
                                    trace=trace, **kw)


def _unshard(res):
    row = np.asarray(res.results[0]["out"], dtype=np.float32).reshape(1, _PRED)
    return np.ascontiguousarray(np.broadcast_to(row, (_B, _PRED)))


def kernel(**inputs) -> np.ndarray:
    return _unshard(_run(inputs, trace=False))
